# revision 5
# baseline (speedup 1.0000x reference)
"""Cost volume (tfa CorrelationCost, kernel_size=1, d=4) on 8 TRN2 cores.

out[b, k, y, x] = (1/C) * sum_c prv[b,c,y,x] * nxt_pad[b,c,y+dy,x+dx],
k = dy*9+dx, dy/dx in 0..8, nxt zero-padded by 4 on each spatial side.

Sharding: core i -> (batch b = i//2, H-half h = i%2). Each core gets the
full-C feature maps for its 64 rows (prv) and 72 padded rows (nxt).

Per-core algorithm (fp16 banded matmul), v2 — HBM-traffic-minimized:
  - pixels are tiled into 8y x 16x = 128 blocks -> lhsT [C=128, 128pix]
  - rhs = the 16y' x 24x' window of UNBANDED padded nxt ([C, 72, 264] in
    SBUF), read via a 2-free-dim access pattern -> no x-band duplication
  - one matmul per tile: psum[pix, n=(dy',dx')]; the 81 useful entries for
    pixel (q,r) are at n = (q+dy)*24 + (r+dx)
  - psum is evacuated (fp32 -> fp16) to a full-384-col SBUF stage,
    alternating vector/scalar engines per y-band
  - out-DMA gathers only the per-q 216-col slabs [24q, 24q+216) that hold
    all 81 useful columns for pixel-row q: 7.08 MB instead of the 12.6 MB
    full dump; the host extracts the banded entries from the slabs.

Traffic per core: prv 4.19 MB + nxt 4.86 MB + out 7.08 MB = 16.1 MB
(vs 23.9 MB for the banded/full-dump v1).

Engine plan: inputs on gpsimd SWDGE (9 nxt row-chunks + 8 prv y-band
chunks, pipelined so matmuls start ~4us in); out slab DMAs on the two
HWDGE rings (SP for vector-evac'd bands, ACT for scalar-evac'd bands,
emitted after that band's evacs so the FIFO never stalls an evac);
dummy ldweights absorb input-DMA waits so each matmul carries only its
psum-slot release wait.
"""

import numpy as np

import concourse.bass as bass
import concourse.tile as tile
from concourse import bacc, mybir
from concourse.bass_utils import run_bass_kernel_spmd

# Problem geometry (hardcoded per spec)
B, C, H, W = 4, 128, 128, 256
D = 4
ND = 2 * D + 1            # 9
K = ND * ND               # 81
HH = H // 2               # 64 rows per core
HP = HH + 2 * D           # 72 padded nxt rows per core
WP = W + 2 * D            # 264 padded nxt cols
YB, XB = 8, 16            # pixel tile: 8 rows x 16 cols = 128 partitions
NY, NX = YB + 2 * D, XB + 2 * D   # 16 x 24 window
NTY, NTX = HH // YB, W // XB      # 8 y-bands x 16 x-tiles
NWIN = NY * NX            # 384
SLAB = ND * NX            # 216 cols per q-slab
N_CORES = 8

F16 = mybir.dt.float16
F32 = mybir.dt.float32


def build_nc():
    nc = bacc.Bacc("TRN2")
    prv_d = nc.declare_dram_parameter("prv_s", [C, NTY * NTX * 128], F16, isOutput=False)
    nxt_d = nc.declare_dram_parameter("nxt_s", [C, HP * WP], F16, isOutput=False)
    out_d = nc.declare_dram_parameter(
        "out_g", [NTY, YB, 16 * NTX * SLAB], F16, isOutput=True
    )

    with tile.TileContext(nc) as tc:
        with (
            tc.tile_pool(name="inp", bufs=1) as inp,
            tc.tile_pool(name="psum", bufs=8, space="PSUM") as pp,
            tc.tile_pool(name="stage", bufs=1) as sp,
        ):
            prv_sb = inp.tile([C, NTY * NTX * 128], F16)
            nxt_sb = inp.tile([C, HP, WP], F16)
            # Stage interleaves psum col c with xb within each y-band:
            # free offset = yb*6144 + c*16 + xb. The out-DMA slab for
            # (yb, q) is then [16 parts, 216*16 contiguous elems] -> 16
            # descriptors of 6912B instead of 256 of 432B (descriptor
            # overhead was the v2 bottleneck).
            stage = sp.tile([128, NTY, NWIN, NTX], F16)

            # Inputs: band yb needs nxt rows [8yb, 8yb+16) (chunks yb,
            # yb+1) and prv chunk yb. The first three chunks go on the
            # HWDGE (sync) ring -- starts in ~0.6us instead of waiting on
            # serialized Q7 SWDGE emission -- the rest via gpsimd SWDGE.
            def nxt_chunk(j, eng):
                eng.dma_start(
                    nxt_sb[:, 8 * j : 8 * j + 8, :],
                    nxt_d[:, 8 * j * WP : (8 * j + 8) * WP],
                )

            def prv_chunk(j, eng):
                lo = j * NTX * 128
                eng.dma_start(
                    prv_sb[:, lo : lo + NTX * 128], prv_d[:, lo : lo + NTX * 128]
                )

            nxt_chunk(0, nc.sync)
            nxt_chunk(1, nc.sync)
            prv_chunk(0, nc.sync)
            for j in range(1, NTY):
                nxt_chunk(j + 1, nc.gpsimd)
                prv_chunk(j, nc.gpsimd)

            for yb in range(NTY):
                # Absorb the input-DMA waits on cheap PE instructions so
                # each matmul below carries only its psum-release wait.
                nc.tensor.ldweights(prv_sb[:, yb * NTX * 128 : yb * NTX * 128 + 1])
                nc.tensor.ldweights(nxt_sb[:, 8 * yb, :1])
                nc.tensor.ldweights(nxt_sb[:, 8 * yb + 8, :1])
                for xb in range(NTX):
                    t = yb * NTX + xb
                    ps = pp.tile([128, NWIN], F32)
                    lhsT = prv_sb[:, t * 128 : (t + 1) * 128]
                    rhs = nxt_sb[:, yb * YB : yb * YB + NY, xb * XB : xb * XB + NX]
                    nc.tensor.matmul(ps, lhsT, rhs, start=True, stop=True)
                    dst = stage[:, yb, :, xb]
                    # One evac engine per y-band -> each out-DMA below
                    # waits on a single semaphore; bands alternate engines.
                    if yb % 2 == 0:
                        nc.vector.tensor_copy(dst, ps)
                    else:
                        nc.scalar.copy(dst, ps)
                # Slab gather: partitions 16q..16q+16 only ever need psum
                # cols [24q, 24q+216). Even bands (vector-evac'd) go out on
                # the SP HWDGE ring, odd (scalar-evac'd) on the ACT ring --
                # ACT's FIFO wait is then always already satisfied.
                eng = nc.sync if yb % 2 == 0 else nc.scalar
                for q in range(YB):
                    src = stage[16 * q : 16 * q + 16, yb, 24 * q : 24 * q + SLAB, :]
                    eng.dma_start(out_d[yb, q], src)
    return nc


def make_in_maps(prv: np.ndarray, nxt: np.ndarray) -> list[dict[str, np.ndarray]]:
    prv = np.asarray(prv, dtype=np.float32)
    nxt = np.asarray(nxt, dtype=np.float32)
    nxt_pad = np.zeros((B, C, H + 2 * D, W + 2 * D), np.float32)
    nxt_pad[:, :, D : D + H, D : D + W] = nxt * np.float32(0.125)
    prv_s = prv * np.float32(0.0625)  # 2^-4 * 2^-3 = 1/C
    in_maps = []
    for core in range(N_CORES):
        b, h = divmod(core, 2)
        # prv tile-major, yb-outer: [C, yb, xb, q, r]
        p = prv_s[b, :, h * HH : (h + 1) * HH, :].reshape(C, NTY, YB, NTX, XB)
        p = np.ascontiguousarray(p.transpose(0, 1, 3, 2, 4)).reshape(C, -1)
        # nxt unbanded: [C, 72, 264]
        x = nxt_pad[b, :, h * HH : h * HH + HP, :]
        in_maps.append(
            {
                "prv_s": p.astype(np.float16),
                "nxt_s": np.ascontiguousarray(x).reshape(C, -1).astype(np.float16),
            }
        )
    return in_maps


def extract_core(G: np.ndarray) -> np.ndarray:
    """[NTY, YB, 16*216*NTX] slab dump -> [K, HH, W] fp32.

    G[yb, q, r, c, xb] holds psum col 24q + c of pixel (q, r) in tile
    (yb, xb); displacement k=(dy,dx) lives at c = 24*dy + r + dx.
    """
    G = np.asarray(G).astype(np.float32).reshape(NTY, YB, 16, SLAB, NTX)
    G = G.transpose(0, 1, 4, 2, 3)                    # [yb, q, xb, r, c]
    dy, dx = np.divmod(np.arange(K), ND)              # [81]
    r = np.arange(XB)
    I2 = 24 * dy[:, None] + r[None, :] + dx[:, None]  # [81, 16]
    T = G[:, :, :, r[None, :], I2]                    # [yb, q, xb, 81, 16]
    T = T.transpose(3, 0, 1, 2, 4)                    # [81, yb, q, xb, r]
    return T.reshape(K, HH, W)


def run(prv: np.ndarray, nxt: np.ndarray, trace: bool = False):
    nc = build_nc()
    nc.finalize()
    in_maps = make_in_maps(prv, nxt)
    res = run_bass_kernel_spmd(nc, in_maps, list(range(N_CORES)), trace=trace)
    out = np.empty((B, K, H, W), np.float32)
    for core in range(N_CORES):
        b, h = divmod(core, 2)
        out[b, :, h * HH : (h + 1) * HH, :] = extract_core(
            res.results[core]["out_g"]
        )
    return out, res


def kernel(prv: np.ndarray, nxt: np.ndarray) -> np.ndarray:
    out, _ = run(prv, nxt, trace=False)
    return out


if __name__ == "__main__":
    rng = np.random.default_rng(0)
    prv = rng.standard_normal((B, C, H, W), dtype=np.float32)
    nxt = rng.standard_normal((B, C, H, W), dtype=np.float32)
    out = kernel(prv, nxt)
    print(out.shape, out.dtype)


# revision 6
# speedup vs baseline: 1.7535x; 1.7535x over previous
"""Cost volume (tfa CorrelationCost, kernel_size=1, d=4) on 8 TRN2 cores.

out[b, k, y, x] = (1/C) * sum_c prv[b,c,y,x] * nxt_pad[b,c,y+dy,x+dx],
k = dy*9+dx, dy/dx in 0..8, nxt zero-padded by 4 on each spatial side.

Sharding: core i -> (batch b = i//2, H-half h = i%2). Each core gets the
full-C feature maps for its 64 rows (prv) and 72 padded rows (nxt).

Per-core algorithm (fp16 banded matmul), v4 — HBM-traffic-minimized:
  - pixels are tiled into 16y x 8x = 128 blocks -> lhsT [C=128, 128pix]
  - rhs = the 24y' x 16x' window of UNBANDED padded nxt ([C, 72, 264] in
    SBUF) via a 2-free-dim access pattern -> no x-band duplication
  - one matmul per tile: psum[pix, n=wy*16+wx]; pixel (q,r) (q=row 0..15,
    r=col 0..7, partition m=8q+r) needs n=(q+dy)*16+(r+dx), i.e. window
    rows wy in [q, q+9) only -> its 81 useful values live in a 9-row slab
  - evac (fp32->fp16, vector/scalar alternating per y-band) writes stage
    interleaved at wy-block granularity: stage[part, yb, wy, xb, wx], so
    each evac writes 24 near-contiguous 32B runs (fast) while the
    out-DMA slab for (yb, q) = wy-rows [q, q+9) x all xb x wx is ONE
    contiguous 9216B run per partition (full DMA line rate)
  - out traffic: 128pix * 9*16 cols * 2B = 4.72 MB vs 12.6 MB full dump.

Traffic per core: prv 4.19 MB + nxt 4.86 MB + out 4.72 MB = 13.8 MB
(vs 23.9 MB v1) -> ~38.5 us at the 358 GB/s HBM-per-core roofline.

Engine plan: first-band inputs on the sync HWDGE ring (starts ~0.6us),
the rest via gpsimd SWDGE; out slab DMAs on the two HWDGE rings (SP for
vector-evac'd bands, ACT for scalar-evac'd bands, emitted after that
band's evacs so the FIFO never stalls an evac); dummy ldweights absorb
input-DMA waits so each matmul carries only its psum-slot release wait.
"""

import numpy as np

import concourse.bass as bass
import concourse.tile as tile
from concourse import bacc, mybir
from concourse.bass_utils import run_bass_kernel_spmd

# Problem geometry (hardcoded per spec)
B, C, H, W = 4, 128, 128, 256
D = 4
ND = 2 * D + 1            # 9
K = ND * ND               # 81
HH = H // 2               # 64 rows per core
HP = HH + 2 * D           # 72 padded nxt rows per core
WP = W + 2 * D            # 264 padded nxt cols
YB, XB = 16, 8            # pixel tile: 16 rows x 8 cols = 128 partitions
NY, NX = YB + 2 * D, XB + 2 * D   # 24 x 16 window
NTY, NTX = HH // YB, W // XB      # 4 y-bands x 32 x-tiles
NWIN = NY * NX            # 384
SLAB = ND * NX            # 144 useful cols per pixel-row q
N_CORES = 8

F16 = mybir.dt.float16
F32 = mybir.dt.float32


def build_nc():
    nc = bacc.Bacc("TRN2")
    prv_d = nc.declare_dram_parameter("prv_s", [C, NTY * NTX * 128], F16, isOutput=False)
    nxt_d = nc.declare_dram_parameter("nxt_s", [C, HP * WP], F16, isOutput=False)
    out_d = nc.declare_dram_parameter(
        "out_g", [NTY, YB, XB * ND * NTX * NX], F16, isOutput=True
    )

    with tile.TileContext(nc) as tc:
        with (
            tc.tile_pool(name="inp", bufs=1) as inp,
            tc.tile_pool(name="psum", bufs=8, space="PSUM") as pp,
            tc.tile_pool(name="stage", bufs=1) as sp,
        ):
            prv_sb = inp.tile([C, NTY * NTX * 128], F16)
            nxt_sb = inp.tile([C, HP, WP], F16)
            # wy-block-interleaved stage: [part, yb, wy, xb, wx]. Evac of
            # tile (yb, xb) writes 24 runs of 16 elems (32B, stride 512
            # elems); the (yb, q) out-slab is wy in [q, q+9) -> one
            # contiguous 9*32*16 = 4608-elem (9216B) run per partition.
            stage = sp.tile([128, NTY, NY, NTX, NX], F16)

            # Inputs: band yb needs nxt rows [16yb, 16yb+24) (8-row chunks
            # 2yb..2yb+2) and prv half-band chunks. First-band deps go on
            # the HWDGE (sync) ring -- no serialized Q7 emission -- the
            # rest via gpsimd SWDGE.
            def nxt_chunk(j, eng):
                eng.dma_start(
                    nxt_sb[:, 8 * j : 8 * j + 8, :],
                    nxt_d[:, 8 * j * WP : (8 * j + 8) * WP],
                )

            def prv_chunk(j, eng):  # half-band chunks of 16 tiles
                lo = j * 16 * 128
                eng.dma_start(
                    prv_sb[:, lo : lo + 16 * 128], prv_d[:, lo : lo + 16 * 128]
                )

            nxt_chunk(0, nc.sync)
            nxt_chunk(1, nc.sync)
            prv_chunk(0, nc.sync)
            nxt_chunk(2, nc.sync)
            prv_chunk(1, nc.gpsimd)
            for yb in range(1, NTY):
                nxt_chunk(2 * yb + 1, nc.gpsimd)
                nxt_chunk(2 * yb + 2, nc.gpsimd)
                prv_chunk(2 * yb, nc.gpsimd)
                prv_chunk(2 * yb + 1, nc.gpsimd)

            for yb in range(NTY):
                # Absorb input-DMA waits on cheap PE instructions so each
                # matmul below carries only its psum-release wait.
                nc.tensor.ldweights(prv_sb[:, yb * NTX * 128 : yb * NTX * 128 + 1])
                for j in range(3):
                    nc.tensor.ldweights(nxt_sb[:, 8 * (2 * yb + j), :1])
                for xb in range(NTX):
                    if xb == NTX // 2:
                        lo = (2 * yb + 1) * 16 * 128
                        nc.tensor.ldweights(prv_sb[:, lo : lo + 1])
                    t = yb * NTX + xb
                    ps = pp.tile([128, NWIN], F32)
                    lhsT = prv_sb[:, t * 128 : (t + 1) * 128]
                    rhs = nxt_sb[:, yb * YB : yb * YB + NY, xb * XB : xb * XB + NX]
                    nc.tensor.matmul(ps, lhsT, rhs, start=True, stop=True)
                    dst = stage[:, yb, :, xb, :]
                    # One evac engine per y-band -> each out-DMA below
                    # waits on a single semaphore; bands alternate engines.
                    if yb % 2 == 0:
                        nc.vector.tensor_copy(dst, ps)
                    else:
                        nc.scalar.copy(dst, ps)
                # Slab gather: pixel-row q only needs window rows
                # [q, q+9). Even bands (vector-evac'd) go out on the SP
                # HWDGE ring, odd (scalar-evac'd) on the ACT ring -- ACT's
                # FIFO wait is then always already satisfied.
                eng = nc.sync if yb % 2 == 0 else nc.scalar
                for q in range(YB):
                    src = stage[XB * q : XB * q + XB, yb, q : q + ND, :, :]
                    eng.dma_start(out_d[yb, q], src)
    return nc


def make_in_maps(prv: np.ndarray, nxt: np.ndarray) -> list[dict[str, np.ndarray]]:
    prv = np.asarray(prv, dtype=np.float32)
    nxt = np.asarray(nxt, dtype=np.float32)
    nxt_pad = np.zeros((B, C, H + 2 * D, W + 2 * D), np.float32)
    nxt_pad[:, :, D : D + H, D : D + W] = nxt * np.float32(0.125)
    prv_s = prv * np.float32(0.0625)  # 2^-4 * 2^-3 = 1/C
    in_maps = []
    for core in range(N_CORES):
        b, h = divmod(core, 2)
        # prv tile-major, yb-outer: [C, yb, xb, q, r]
        p = prv_s[b, :, h * HH : (h + 1) * HH, :].reshape(C, NTY, YB, NTX, XB)
        p = np.ascontiguousarray(p.transpose(0, 1, 3, 2, 4)).reshape(C, -1)
        # nxt unbanded: [C, 72, 264]
        x = nxt_pad[b, :, h * HH : h * HH + HP, :]
        in_maps.append(
            {
                "prv_s": p.astype(np.float16),
                "nxt_s": np.ascontiguousarray(x).reshape(C, -1).astype(np.float16),
            }
        )
    return in_maps


def extract_core(G: np.ndarray) -> np.ndarray:
    """[NTY, YB, XB*9*NTX*NX] slab dump -> [K, HH, W] fp32.

    G[yb, q, r, j, xb, wx] holds psum col (q+j)*NX + wx of pixel (q, r)
    in tile (yb, xb); displacement k=(dy,dx) is at j=dy, wx = r + dx.
    """
    G = np.asarray(G).astype(np.float32).reshape(NTY, YB, XB, ND, NTX, NX)
    G = G.transpose(0, 1, 4, 2, 3, 5)                 # [yb, q, xb, r, j, wx]
    dy, dx = np.divmod(np.arange(K), ND)              # [81]
    r = np.arange(XB)
    ridx = np.broadcast_to(r[None, :], (K, XB))       # [81, 8]
    jidx = np.broadcast_to(dy[:, None], (K, XB))      # [81, 8]
    wxidx = r[None, :] + dx[:, None]                  # [81, 8]
    T = G[:, :, :, ridx, jidx, wxidx]                 # [yb, q, xb, 81, r]
    T = T.transpose(3, 0, 1, 2, 4)                    # [81, yb, q, xb, r]
    return T.reshape(K, HH, W)


def run(prv: np.ndarray, nxt: np.ndarray, trace: bool = False):
    nc = build_nc()
    nc.finalize()
    in_maps = make_in_maps(prv, nxt)
    res = run_bass_kernel_spmd(nc, in_maps, list(range(N_CORES)), trace=trace)
    out = np.empty((B, K, H, W), np.float32)
    for core in range(N_CORES):
        b, h = divmod(core, 2)
        out[b, :, h * HH : (h + 1) * HH, :] = extract_core(
            res.results[core]["out_g"]
        )
    return out, res


def kernel(prv: np.ndarray, nxt: np.ndarray) -> np.ndarray:
    out, _ = run(prv, nxt, trace=False)
    return out


if __name__ == "__main__":
    rng = np.random.default_rng(0)
    prv = rng.standard_normal((B, C, H, W), dtype=np.float32)
    nxt = rng.standard_normal((B, C, H, W), dtype=np.float32)
    out = kernel(prv, nxt)
    print(out.shape, out.dtype)
